# revision 37
# baseline (speedup 1.0000x reference)
"""Bass/Trainium2 kernel for nn_CTRGC (CTR-GC graph conv block).

Sharding: data-parallel over batch N=64 across 8 cores (8 samples/core).
All weights/router params are host-folded and replicated.

Per-core pipeline (per sample, C=128 partitions, T=64, V=25, O=128, R=16):
  - tada matmul Y[c,(t,v)] = tada_w.T @ x on PE; x3 = Y * alpha_rf fused
    into the PSUM->SBUF drain (DVE), written v32-padded.
  - router MLP (alpha_rf) and m (topology) exactly as the reference, all
    tiny PE matmuls + ACT bias/activations; m written v32-padded.
  - DVE 32x32 StreamTranspose puts x3 and m in v-on-partition layout:
      X3v[32a+v, 32t+cl] = x3[32a+cl, t, v]
      mv [32a+v, 32u+cl] = m [32a+cl, u, v]
  - graph conv out[c,t,u] = sum_v m[c,u,v] x3[c,t,v] as 128 small PE
    matmuls (K=32, M=25, N=64), 16 concurrently via tile_position
    (row group a = channel block, col group j), 8 waves of 16 channels.
  - PSUM waves -> CT collection (ACT copies), DVE StreamTranspose back to
    channel-major, ACT compaction to (t,u), DMA out with a channel
    permutation folded into the DRAM access pattern.
"""

import numpy as np

N_CORES = 8
N, C, T, V = 64, 128, 64, 25
O, R, CH = 128, 16, 64
NLOC = N // N_CORES
TV = T * V          # 1600
UV = V * V          # 625
V32 = 32
TV32 = T * V32      # 2048
U32V32 = V32 * V32  # 1024
BN_EPS = 1e-5

_CACHE = {}


def _build_program(reps=1):
    """Build the per-core program. reps>1 repeats the whole computation
    (same inputs/outputs) inside one NEFF — used by test.py to measure
    steady-state HW exec time via slope, cancelling host dispatch latency."""
    import concourse.bacc as bacc
    import concourse.tile as tile
    import concourse.mybir as mybir

    f32 = mybir.dt.float32
    f32r = mybir.dt.float32r
    bf16 = mybir.dt.bfloat16
    AX = mybir.AxisListType
    ALU = mybir.AluOpType
    ACT = mybir.ActivationFunctionType

    nc = bacc.Bacc("TRN2", target_bir_lowering=False, debug=False,
                   num_devices=N_CORES)

    # ---- DRAM I/O ----
    xs = nc.dram_tensor("xs", [NLOC, C, T, V], f32, kind="ExternalInput").ap()
    out = nc.dram_tensor("out", [NLOC, O, T, V], f32, kind="ExternalOutput").ap()

    w_names = {
        "wT_tada": [C, O],
        "rf_gT": [C, C],
        "rf_g_b": [C, 1],
        "w1T": [C, R],
        "b1": [R, 1],
        "w2T": [C, R],
        "b2": [R, 1],
        "rf_aT": [C, 3 * CH],
        "rf_ab": [CH, 1],
        "rf_bT": [CH, 3 * O],
        "lhsT18": [R + 2, O],
        "d18c": [2, UV],
    }
    wd = {k: nc.dram_tensor(k, s, f32, kind="ExternalInput").ap()
          for k, s in w_names.items()}

    with tile.TileContext(nc) as tc:
        with (
            tc.tile_pool(name="weights", bufs=1) as wpool,
            tc.tile_pool(name="xin", bufs=2) as xpool,
            tc.tile_pool(name="xbf", bufs=2) as xbfpool,
            tc.tile_pool(name="x3p", bufs=2) as x3pool,
            tc.tile_pool(name="x3vp", bufs=2) as x3vpool,
            tc.tile_pool(name="mp", bufs=2) as mpool,
            tc.tile_pool(name="mvp", bufs=2) as mvpool,
            tc.tile_pool(name="ctp", bufs=2) as ctpool,
            tc.tile_pool(name="o32p", bufs=2) as o32pool,
            tc.tile_pool(name="outp", bufs=2) as opool,
            tc.tile_pool(name="small", bufs=3) as spool,
            tc.tile_pool(name="d18p", bufs=2) as dpool,
            tc.tile_pool(name="psC", bufs=2, space="PSUM") as psc,
            tc.tile_pool(name="psP", bufs=1, space="PSUM") as psp,
            tc.tile_pool(name="psS", bufs=2, space="PSUM") as pss,
        ):
            # ---- load weights once ----
            w = {}
            for k, s in w_names.items():
                w[k] = wpool.tile(s, f32, tag=k, name=k)
                nc.sync.dma_start(w[k][:], wd[k])
            wt_bf = wpool.tile([C, O], bf16, tag="wt_bf", name="wt_bf")
            nc.scalar.copy(wt_bf[:], w["wT_tada"][:])

            def emit_waves(X3v, mv, n_b):
                """Stage B: graph-conv wave matmuls + PSUM collection.
                Emitted at the top of the NEXT iteration so the PE starts an
                iteration with ready work while the DVE runs reductions.

                tile (a, j), wave w -> channel c = 32a + 4w + j
                P_a[32j+u, 64w+t] = out[c, t, u]"""
                P = [psp.tile([C, 512], f32, tag=f"P{a}", name=f"P{a}")
                     for a in range(4)]
                for wv in range(8):
                    for a in range(4):
                        for j in range(4):
                            cl = 4 * wv + j
                            lhsT = mv[:].rearrange(
                                "p (u cl) -> p u cl", cl=V32
                            )[32 * a:32 * a + 32, :, cl]
                            rhs = X3v[:].rearrange(
                                "p (t cl) -> p t cl", cl=V32
                            )[32 * a:32 * a + 32, :, cl]
                            nc.tensor.matmul(
                                P[a][32 * j:32 * j + 32, 64 * wv:64 * wv + 64],
                                lhsT, rhs, start=True, stop=True,
                                tile_position=(32 * a, 32 * j))
                # collect: CT[32j+u, 32t + (4w+a)] = P_a[32j+u, 64w+t]
                CT = ctpool.tile([C, TV32], f32, tag="CT", name="CT")
                for a in range(4):
                    nc.scalar.copy(
                        CT[:].rearrange("p (t cl) -> p t cl", cl=V32)
                        [:, :, a::4].rearrange("p t w -> p w t"),
                        P[a][:].rearrange("p (w t) -> p w t", t=64))
                return CT

            def emit_tail(CT_prev, n_prev):
                """Stage C: back-transpose, compact and store a finished
                sample — two iterations behind stage A."""
                O32 = o32pool.tile([C, TV32], f32, tag="O32", name="O32")
                nc.vector.transpose(O32[:], CT_prev[:])
                OUTC = opool.tile([O, TV], f32, tag="OUTC", name="OUTC")
                nc.scalar.copy(
                    OUTC[:].rearrange("p (t u) -> p t u", u=V),
                    O32[:].rearrange("p (t u) -> p t u", u=V32)[:, :, 0:V])
                # partition p = 32j + 4w + a holds channel c = 32a + 4w + j
                for j in range(4):
                    nc.sync.dma_start(
                        out[n_prev].rearrange("(a w j) t v -> j w a (t v)",
                                              a=4, w=8, j=4)[j],
                        OUTC[32 * j:32 * (j + 1), :])

            pend_b = None   # (X3v, mv, n) waiting for stage B
            pend_c = None   # (CT, n) waiting for stage C
            for it in range(reps * NLOC):
                n = it % NLOC

                # ---- stage B of the previous sample (PE-ready work first) ----
                new_c = None
                if pend_b is not None:
                    new_c = (emit_waves(*pend_b), pend_b[2])
                    pend_b = None


                # ---- load x[n] ----
                X = xpool.tile([C, TV], f32, tag="X", name="X")
                nc.sync.dma_start(X[:], xs[n].rearrange("c t v -> c (t v)"))

                # ---- reductions ----
                xt_sum = spool.tile([C, V], f32, tag="xt_sum", name="xt_sum")
                nc.vector.tensor_reduce(
                    xt_sum[:], X[:].rearrange("c (t v) -> c v t", v=V),
                    axis=AX.X, op=ALU.add)
                xa_sum = spool.tile([C, T], f32, tag="xa_sum", name="xa_sum")
                nc.vector.tensor_reduce(
                    xa_sum[:], X[:].rearrange("c (t v) -> c t v", v=V),
                    axis=AX.X, op=ALU.add)
                g_sum = spool.tile([C, 1], f32, tag="g_sum", name="g_sum")
                nc.vector.tensor_reduce(g_sum[:], xa_sum[:], axis=AX.X,
                                        op=ALU.add)

                # ---- router: g2 = rf_g_w @ g + rf_g_b ----
                g2_ps = pss.tile([C, 64], f32, tag="ps_small", name="ps_small")
                nc.tensor.matmul(g2_ps[:, 0:1], w["rf_gT"][:], g_sum[:],
                                 start=True, stop=True)
                g2 = spool.tile([C, 1], f32, tag="g2", name="g2")
                nc.scalar.activation(g2[:], g2_ps[:, 0:1], ACT.Identity,
                                     bias=w["rf_g_b"][:])

                # ---- xa = xa_sum/V + g2 (padded to 66 cols for 3-tap conv) ----
                xa = spool.tile([C, T + 2], f32, tag="xa", name="xa")
                nc.vector.memset(xa[:, 0:1], 0.0)
                nc.vector.memset(xa[:, T + 1:T + 2], 0.0)
                nc.vector.scalar_tensor_tensor(
                    xa[:, 1:T + 1], xa_sum[:], 1.0 / V,
                    g2[:].broadcast_to((C, T)), op0=ALU.mult, op1=ALU.add)

                # ---- a = relu(bn(conv1d(xa, rf_a))) ----
                a_ps = pss.tile([CH, 64], f32, tag="ps_small", name="ps_small")
                for k in range(3):
                    nc.tensor.matmul(a_ps[:, 0:T], w["rf_aT"][:, k * CH:(k + 1) * CH],
                                     xa[:, k:k + T], start=(k == 0), stop=(k == 2))
                a_pad = spool.tile([CH, T + 2], f32, tag="a_pad", name="a_pad")
                nc.vector.memset(a_pad[:, 0:1], 0.0)
                nc.vector.memset(a_pad[:, T + 1:T + 2], 0.0)
                nc.scalar.activation(a_pad[:, 1:T + 1], a_ps[:, 0:T], ACT.Relu,
                                     bias=w["rf_ab"][:])

                # ---- alpha_rf = conv1d(a, rf_b) + 1 ----
                arf_ps = pss.tile([O, 64], f32, tag="ps_small", name="ps_small")
                for k in range(3):
                    nc.tensor.matmul(arf_ps[:, 0:T], w["rf_bT"][:, k * O:(k + 1) * O],
                                     a_pad[:, k:k + T], start=(k == 0), stop=(k == 2))
                alpha_rf = spool.tile([O, T], f32, tag="alpha_rf", name="alpha_rf")
                nc.scalar.activation(alpha_rf[:], arf_ps[:, 0:T], ACT.Identity,
                                     bias=1.0)

                # ---- x1/x2 = conv1/2(xt) ----
                x1_ps = pss.tile([R, 64], f32, tag="ps_small", name="ps_small")
                nc.tensor.matmul(x1_ps[:, 0:V], w["w1T"][:], xt_sum[:],
                                 start=True, stop=True)
                x2_ps = pss.tile([R, 64], f32, tag="ps_small", name="ps_small")
                nc.tensor.matmul(x2_ps[:, 0:V], w["w2T"][:], xt_sum[:],
                                 start=True, stop=True)
                x1 = spool.tile([R, V], f32, tag="x1", name="x1")
                nc.scalar.activation(x1[:], x1_ps[:, 0:V], ACT.Identity,
                                     bias=w["b1"][:])
                x2 = spool.tile([R, V], f32, tag="x2", name="x2")
                nc.scalar.activation(x2[:], x2_ps[:, 0:V], ACT.Identity,
                                     bias=w["b2"][:])

                # ---- D18 = [tanh(x1[u]-x2[v]); ones; A] ----
                D18 = dpool.tile([R + 2, UV], f32, tag="D18", name="D18")
                nc.sync.dma_start(D18[R:R + 2, :], wd["d18c"])
                nc.vector.tensor_tensor(
                    D18[0:R, :].rearrange("r (u v) -> r u v", v=V),
                    x1[:].unsqueeze(2).broadcast_to((R, V, V)),
                    x2[:].unsqueeze(1).broadcast_to((R, V, V)),
                    op=ALU.subtract)
                nc.scalar.activation(D18[0:R, :], D18[0:R, :], ACT.Tanh)

                # ---- tada matmul in t-aligned chunks; x3 = Y*alpha_rf drain ----
                # bf16 runs the PE at 1 cycle/row (vs 4 for fp32) and enables
                # fast weight load; ACT converts X (it has slack, DVE doesn't)
                Xbf = xbfpool.tile([C, TV], bf16, tag="Xbf", name="Xbf")
                nc.scalar.copy(Xbf[:], X[:])
                x3 = x3pool.tile([O, TV32], bf16, tag="x3", name="x3")
                nc.gpsimd.memset(x3[:], 0.0)  # zeroes the v32 pads
                TC = 16 * V  # 400 cols = 16 t's per chunk
                for k in range(4):
                    Yc = psc.tile([C, TC], f32, tag="chunk", name="chunk")
                    nc.tensor.matmul(Yc[:], wt_bf[:],
                                     Xbf[:, k * TC:(k + 1) * TC],
                                     start=True, stop=True)
                    t0, t1 = 16 * k, 16 * (k + 1)
                    nc.vector.tensor_tensor(
                        x3[:].rearrange("c (t v) -> c t v", v=V32)[:, t0:t1, 0:V],
                        Yc[:].rearrange("c (t v) -> c t v", v=V),
                        alpha_rf[:, t0:t1].unsqueeze(2).broadcast_to((O, 16, V)),
                        op=ALU.mult)
                X3v = x3vpool.tile([C, TV32], bf16, tag="X3v", name="X3v")
                nc.vector.transpose(X3v[:], x3[:])

                # ---- m[c, (u,v)] = alpha*(conv4 @ D + b4) + A ----
                # m32 is (u32, v32)-padded: pad u-slots/v-lanes are zero so the
                # graph-conv matmuls can use full 32x32 weight tiles (the pad
                # rows of each PSUM block compute to zero, keeping every read
                # byte defined).
                m32 = mpool.tile([O, U32V32], bf16, tag="m32", name="m32")
                nc.gpsimd.memset(m32[:], 0.0)
                for (u0, u1) in ((0, 16), (16, V)):
                    mc = psc.tile([C, TC], f32, tag="chunk", name="chunk")
                    nc.tensor.matmul(mc[:, 0:(u1 - u0) * V], w["lhsT18"][:],
                                     D18[:, u0 * V:u1 * V], start=True, stop=True)
                    nc.scalar.copy(
                        m32[:].rearrange("c (u v) -> c u v", v=V32)
                        [:, u0:u1, 0:V],
                        mc[:, 0:(u1 - u0) * V].rearrange("c (u v) -> c u v", v=V))
                mv = mvpool.tile([C, U32V32], bf16, tag="mv", name="mv")
                nc.vector.transpose(mv[:], m32[:])
                pend_b = (X3v, mv, n)

                # ---- stage C: finish the sample from two iterations back ----
                if pend_c is not None:
                    emit_tail(*pend_c)
                pend_c = new_c

            # drain the pipeline
            new_c = (emit_waves(*pend_b), pend_b[2])
            if pend_c is not None:
                emit_tail(*pend_c)
            emit_tail(*new_c)

    nc.compile()
    return nc


def _fold_weights(A, conv1_w, conv1_b, conv2_w, conv2_b, conv4_w, conv4_b,
                  rf_g_w, rf_g_b, rf_a_w, rf_a_b, bn_gamma, bn_beta,
                  rf_b_w, tada_w, alpha):
    af = float(np.asarray(alpha))
    f = np.float32
    s = (bn_gamma / np.sqrt(1.0 + BN_EPS)).astype(f)
    rf_a_w2 = (rf_a_w * s[:, None, None]).astype(f)
    rf_ab2 = (rf_a_b * s + bn_beta).astype(f)
    lhsT18 = np.concatenate([
        af * conv4_w.T.astype(f),            # (16, 128)
        af * conv4_b[None, :].astype(f),     # (1, 128)
        np.ones((1, O), f),
    ], axis=0)
    d18c = np.stack([np.ones(UV, f), A.astype(f).reshape(UV)], axis=0)
    return {
        "wT_tada": np.ascontiguousarray(tada_w.T.astype(f)),
        "rf_gT": np.ascontiguousarray((rf_g_w.T / (T * V)).astype(f)),
        "rf_g_b": rf_g_b.astype(f).reshape(C, 1),
        "w1T": np.ascontiguousarray((conv1_w.T / T).astype(f)),
        "b1": conv1_b.astype(f).reshape(R, 1),
        "w2T": np.ascontiguousarray((conv2_w.T / T).astype(f)),
        "b2": conv2_b.astype(f).reshape(R, 1),
        "rf_aT": np.concatenate([rf_a_w2[:, :, k].T for k in range(3)], axis=1),
        "rf_ab": rf_ab2.reshape(CH, 1),
        "rf_bT": np.concatenate([rf_b_w[:, :, k].T.astype(f) for k in range(3)],
                                axis=1),
        "lhsT18": lhsT18,
        "d18c": d18c,
    }


def _neff_io(nc):
    """(in_names, out_names, out_avals, out_shapes, partition_name)."""
    import jax
    import concourse.mybir as mybir

    partition_name = (nc.partition_id_tensor.name
                      if nc.partition_id_tensor else None)
    in_names, out_names, out_avals, out_shapes = [], [], [], []
    for alloc in nc.m.functions[0].allocations:
        if not isinstance(alloc, mybir.MemoryLocationSet):
            continue
        name = alloc.memorylocations[0].name
        if alloc.kind == "ExternalInput":
            if name != partition_name:
                in_names.append(name)
        elif alloc.kind == "ExternalOutput":
            out_names.append(name)
            shape = tuple(alloc.tensor_shape)
            dtype = mybir.dt.np(alloc.dtype)
            out_avals.append(jax.core.ShapedArray(shape, dtype))
            out_shapes.append((shape, dtype))
    return in_names, out_names, out_avals, out_shapes, partition_name


def _make_runner(nc):
    """Cached jitted SPMD executable (mirrors bass2jax.run_bass_via_pjrt)."""
    import jax
    from jax.sharding import Mesh, PartitionSpec
    from jax.experimental.shard_map import shard_map
    from concourse import bass2jax

    bass2jax.install_neuronx_cc_hook()
    assert nc.dbg_addr is None
    in_names, out_names, out_avals, out_shapes, partition_name = _neff_io(nc)
    n_params = len(in_names)
    all_in_names = tuple(in_names) + tuple(out_names)
    if partition_name is not None:
        all_in_names = all_in_names + (partition_name,)

    def _body(*args):
        operands = list(args)
        if partition_name is not None:
            operands.append(bass2jax.partition_id_tensor())
        outs = bass2jax._bass_exec_p.bind(
            *operands, out_avals=tuple(out_avals), in_names=all_in_names,
            out_names=tuple(out_names), lowering_input_output_aliases=(),
            sim_require_finite=True, sim_require_nnan=True, nc=nc)
        return tuple(outs)

    devices = jax.devices()[:N_CORES]
    mesh = Mesh(np.asarray(devices), ("core",))
    n_outs = len(out_names)
    # No donation: the kernel writes every element of every output, so the
    # zero placeholders can stay device-resident and be reused across calls.
    sharded = jax.jit(
        shard_map(_body, mesh=mesh,
                  in_specs=(PartitionSpec("core"),) * (n_params + n_outs),
                  out_specs=(PartitionSpec("core"),) * n_outs,
                  check_rep=False),
        keep_unused=True)
    zeros_dev = [jax.device_put(np.zeros((N_CORES * s[0], *s[1:]), d))
                 for s, d in out_shapes]
    return sharded, in_names, out_names, out_shapes, zeros_dev


def _prepare_concat_inputs(x, wmap, in_names):
    """Global (n_cores*dim0, ...) arrays in the NEFF's input order."""
    per = {"xs": np.ascontiguousarray(x, dtype=np.float32)}
    for k, v in wmap.items():
        per[k] = np.concatenate([v[None]] * N_CORES, axis=0).reshape(
            N_CORES * v.shape[0], *v.shape[1:])
    return [per[nm] for nm in in_names]


def kernel(x, A, conv1_w, conv1_b, conv2_w, conv2_b, conv4_w, conv4_b,
           rf_g_w, rf_g_b, rf_a_w, rf_a_b, bn_gamma, bn_beta,
           rf_b_w, tada_w, alpha):
    if "nc" not in _CACHE:
        _CACHE["nc"] = _build_program()
        _CACHE["runner"] = _make_runner(_CACHE["nc"])
    sharded, in_names, out_names, out_shapes, zeros_dev = _CACHE["runner"]

    wmap = _fold_weights(A, conv1_w, conv1_b, conv2_w, conv2_b, conv4_w,
                         conv4_b, rf_g_w, rf_g_b, rf_a_w, rf_a_b, bn_gamma,
                         bn_beta, rf_b_w, tada_w, alpha)
    wmap = {k: np.ascontiguousarray(v, dtype=np.float32) for k, v in wmap.items()}

    ins = _prepare_concat_inputs(x, wmap, in_names)
    outs = sharded(*ins, *zeros_dev)
    i = out_names.index("out")
    return np.asarray(outs[i])


# revision 40
# speedup vs baseline: 2.3266x; 2.3266x over previous
"""Bass/Trainium2 kernel for nn_CTRGC (CTR-GC graph conv block).

Sharding: data-parallel over batch N=64 across 8 cores (8 samples/core).
All weights/router params are host-folded and replicated.

Per-core pipeline (per sample, C=128 partitions, T=64, V=25, O=128, R=16):
  - tada matmul Y[c,(t,v)] = tada_w.T @ x on PE; x3 = Y * alpha_rf fused
    into the PSUM->SBUF drain (DVE), written v32-padded.
  - router MLP (alpha_rf) and m (topology) exactly as the reference, all
    tiny PE matmuls + ACT bias/activations; m written v32-padded.
  - DVE 32x32 StreamTranspose puts x3 and m in v-on-partition layout:
      X3v[32a+v, 32t+cl] = x3[32a+cl, t, v]
      mv [32a+v, 32u+cl] = m [32a+cl, u, v]
  - graph conv out[c,t,u] = sum_v m[c,u,v] x3[c,t,v] as 128 small PE
    matmuls (K=32, M=25, N=64), 16 concurrently via tile_position
    (row group a = channel block, col group j), 8 waves of 16 channels.
  - PSUM waves -> CT collection (ACT copies), DVE StreamTranspose back to
    channel-major, ACT compaction to (t,u), DMA out with a channel
    permutation folded into the DRAM access pattern.
"""

import numpy as np

N_CORES = 8
N, C, T, V = 64, 128, 64, 25
O, R, CH = 128, 16, 64
NLOC = N // N_CORES
TV = T * V          # 1600
UV = V * V          # 625
V32 = 32
TV32 = T * V32      # 2048
U32V32 = V32 * V32  # 1024
BN_EPS = 1e-5

_CACHE = {}


def _build_program(reps=1):
    """Build the per-core program. reps>1 repeats the whole computation
    (same inputs/outputs) inside one NEFF — used by test.py to measure
    steady-state HW exec time via slope, cancelling host dispatch latency."""
    import concourse.bacc as bacc
    import concourse.tile as tile
    import concourse.mybir as mybir

    f32 = mybir.dt.float32
    f32r = mybir.dt.float32r
    bf16 = mybir.dt.bfloat16
    AX = mybir.AxisListType
    ALU = mybir.AluOpType
    ACT = mybir.ActivationFunctionType

    nc = bacc.Bacc("TRN2", target_bir_lowering=False, debug=False,
                   num_devices=N_CORES)

    # ---- DRAM I/O ----
    xs = nc.dram_tensor("xs", [NLOC, C, T, V], f32, kind="ExternalInput").ap()
    out = nc.dram_tensor("out", [NLOC, O, T, V], f32, kind="ExternalOutput").ap()

    w_names = {
        "wT_tada": [C, O],
        "rf_gT": [C, C],
        "rf_g_b": [C, 1],
        "w1T": [C, R],
        "b1": [R, 1],
        "w2T": [C, R],
        "b2": [R, 1],
        "rf_aT": [C, 3 * CH],
        "rf_ab": [CH, 1],
        "rf_bT": [CH, 3 * O],
        "lhsT18": [R + 2, O],
        "d18c": [2, UV],
    }
    wd = {k: nc.dram_tensor(k, s, f32, kind="ExternalInput").ap()
          for k, s in w_names.items()}

    with tile.TileContext(nc) as tc:
        with (
            tc.tile_pool(name="weights", bufs=1) as wpool,
            tc.tile_pool(name="xin", bufs=2) as xpool,
            tc.tile_pool(name="x3p", bufs=2) as x3pool,
            tc.tile_pool(name="x3vp", bufs=2) as x3vpool,
            tc.tile_pool(name="mp", bufs=2) as mpool,
            tc.tile_pool(name="mvp", bufs=2) as mvpool,
            tc.tile_pool(name="ctp", bufs=2) as ctpool,
            tc.tile_pool(name="o32p", bufs=2) as o32pool,
            tc.tile_pool(name="outp", bufs=2) as opool,
            tc.tile_pool(name="small", bufs=3) as spool,
            tc.tile_pool(name="d18p", bufs=2) as dpool,
            tc.tile_pool(name="psC", bufs=2, space="PSUM") as psc,
            tc.tile_pool(name="psP", bufs=1, space="PSUM") as psp,
            tc.tile_pool(name="psS", bufs=2, space="PSUM") as pss,
        ):
            # ---- load weights once ----
            w = {}
            for k, s in w_names.items():
                w[k] = wpool.tile(s, f32, tag=k, name=k)
                nc.sync.dma_start(w[k][:], wd[k])


            def emit_waves(X3v, mv, n_b):
                """Stage B: graph-conv wave matmuls + PSUM collection.
                Emitted at the top of the NEXT iteration so the PE starts an
                iteration with ready work while the DVE runs reductions.

                tile (a, j), wave w -> channel c = 32a + 4w + j
                P_a[32j+u, 64w+t] = out[c, t, u]"""
                P = [psp.tile([C, 512], f32, tag=f"P{a}", name=f"P{a}")
                     for a in range(4)]
                for wv in range(8):
                    for a in range(4):
                        for j in range(4):
                            cl = 4 * wv + j
                            lhsT = mv[:].rearrange(
                                "p (u cl) -> p u cl", cl=V32
                            )[32 * a:32 * a + 32, :, cl]
                            rhs = X3v[:].rearrange(
                                "p (t cl) -> p t cl", cl=V32
                            )[32 * a:32 * a + 32, :, cl]
                            nc.tensor.matmul(
                                P[a][32 * j:32 * j + 32, 64 * wv:64 * wv + 64],
                                lhsT, rhs, start=True, stop=True,
                                tile_position=(32 * a, 32 * j))
                # collect: CT[32j+u, 32t + (4w+a)] = P_a[32j+u, 64w+t]
                CT = ctpool.tile([C, TV32], f32, tag="CT", name="CT")
                for a in range(4):
                    nc.scalar.copy(
                        CT[:].rearrange("p (t cl) -> p t cl", cl=V32)
                        [:, :, a::4].rearrange("p t w -> p w t"),
                        P[a][:].rearrange("p (w t) -> p w t", t=64))
                return CT

            def emit_tail(CT_prev, n_prev):
                """Stage C: back-transpose, compact and store a finished
                sample — two iterations behind stage A."""
                O32 = o32pool.tile([C, TV32], f32, tag="O32", name="O32")
                nc.vector.transpose(O32[:], CT_prev[:])
                OUTC = opool.tile([O, TV], f32, tag="OUTC", name="OUTC")
                nc.scalar.copy(
                    OUTC[:].rearrange("p (t u) -> p t u", u=V),
                    O32[:].rearrange("p (t u) -> p t u", u=V32)[:, :, 0:V])
                # partition p = 32j + 4w + a holds channel c = 32a + 4w + j
                for j in range(4):
                    nc.sync.dma_start(
                        out[n_prev].rearrange("(a w j) t v -> j w a (t v)",
                                              a=4, w=8, j=4)[j],
                        OUTC[32 * j:32 * (j + 1), :])

            pend_b = None   # (X3v, mv, n) waiting for stage B
            pend_c = None   # (CT, n) waiting for stage C
            for it in range(reps * NLOC):
                n = it % NLOC

                # ---- stage B of the previous sample (PE-ready work first) ----
                new_c = None
                if pend_b is not None:
                    new_c = (emit_waves(*pend_b), pend_b[2])
                    pend_b = None


                # ---- load x[n] ----
                X = xpool.tile([C, TV], f32, tag="X", name="X")
                nc.sync.dma_start(X[:], xs[n].rearrange("c t v -> c (t v)"))

                # ---- reductions ----
                xt_sum = spool.tile([C, V], f32, tag="xt_sum", name="xt_sum")
                nc.vector.tensor_reduce(
                    xt_sum[:], X[:].rearrange("c (t v) -> c v t", v=V),
                    axis=AX.X, op=ALU.add)
                xa_sum = spool.tile([C, T], f32, tag="xa_sum", name="xa_sum")
                nc.vector.tensor_reduce(
                    xa_sum[:], X[:].rearrange("c (t v) -> c t v", v=V),
                    axis=AX.X, op=ALU.add)
                g_sum = spool.tile([C, 1], f32, tag="g_sum", name="g_sum")
                nc.vector.tensor_reduce(g_sum[:], xa_sum[:], axis=AX.X,
                                        op=ALU.add)

                # ---- router: g2 = rf_g_w @ g + rf_g_b ----
                g2_ps = pss.tile([C, 64], f32, tag="ps_small", name="ps_small")
                nc.tensor.matmul(g2_ps[:, 0:1], w["rf_gT"][:], g_sum[:],
                                 start=True, stop=True)
                g2 = spool.tile([C, 1], f32, tag="g2", name="g2")
                nc.scalar.activation(g2[:], g2_ps[:, 0:1], ACT.Identity,
                                     bias=w["rf_g_b"][:])

                # ---- xa = xa_sum/V + g2 (padded to 66 cols for 3-tap conv) ----
                xa = spool.tile([C, T + 2], f32, tag="xa", name="xa")
                nc.vector.memset(xa[:, 0:1], 0.0)
                nc.vector.memset(xa[:, T + 1:T + 2], 0.0)
                nc.vector.scalar_tensor_tensor(
                    xa[:, 1:T + 1], xa_sum[:], 1.0 / V,
                    g2[:].broadcast_to((C, T)), op0=ALU.mult, op1=ALU.add)

                # ---- a = relu(bn(conv1d(xa, rf_a))) ----
                a_ps = pss.tile([CH, 64], f32, tag="ps_small", name="ps_small")
                for k in range(3):
                    nc.tensor.matmul(a_ps[:, 0:T], w["rf_aT"][:, k * CH:(k + 1) * CH],
                                     xa[:, k:k + T], start=(k == 0), stop=(k == 2))
                a_pad = spool.tile([CH, T + 2], f32, tag="a_pad", name="a_pad")
                nc.vector.memset(a_pad[:, 0:1], 0.0)
                nc.vector.memset(a_pad[:, T + 1:T + 2], 0.0)
                nc.scalar.activation(a_pad[:, 1:T + 1], a_ps[:, 0:T], ACT.Relu,
                                     bias=w["rf_ab"][:])

                # ---- alpha_rf = conv1d(a, rf_b) + 1 ----
                arf_ps = pss.tile([O, 64], f32, tag="ps_small", name="ps_small")
                for k in range(3):
                    nc.tensor.matmul(arf_ps[:, 0:T], w["rf_bT"][:, k * O:(k + 1) * O],
                                     a_pad[:, k:k + T], start=(k == 0), stop=(k == 2))
                alpha_rf = spool.tile([O, T], f32, tag="alpha_rf", name="alpha_rf")
                nc.scalar.activation(alpha_rf[:], arf_ps[:, 0:T], ACT.Identity,
                                     bias=1.0)

                # ---- x1/x2 = conv1/2(xt) ----
                x1_ps = pss.tile([R, 64], f32, tag="ps_small", name="ps_small")
                nc.tensor.matmul(x1_ps[:, 0:V], w["w1T"][:], xt_sum[:],
                                 start=True, stop=True)
                x2_ps = pss.tile([R, 64], f32, tag="ps_small", name="ps_small")
                nc.tensor.matmul(x2_ps[:, 0:V], w["w2T"][:], xt_sum[:],
                                 start=True, stop=True)
                x1 = spool.tile([R, V], f32, tag="x1", name="x1")
                nc.scalar.activation(x1[:], x1_ps[:, 0:V], ACT.Identity,
                                     bias=w["b1"][:])
                x2 = spool.tile([R, V], f32, tag="x2", name="x2")
                nc.scalar.activation(x2[:], x2_ps[:, 0:V], ACT.Identity,
                                     bias=w["b2"][:])

                # ---- D18 = [tanh(x1[u]-x2[v]); ones; A] ----
                D18 = dpool.tile([R + 2, UV], f32, tag="D18", name="D18")
                nc.sync.dma_start(D18[R:R + 2, :], wd["d18c"])
                nc.vector.tensor_tensor(
                    D18[0:R, :].rearrange("r (u v) -> r u v", v=V),
                    x1[:].unsqueeze(2).broadcast_to((R, V, V)),
                    x2[:].unsqueeze(1).broadcast_to((R, V, V)),
                    op=ALU.subtract)
                nc.scalar.activation(D18[0:R, :], D18[0:R, :], ACT.Tanh)

                # ---- tada matmul in t-aligned chunks; x3 = Y*alpha_rf drain ----
                x3 = x3pool.tile([O, TV32], f32, tag="x3", name="x3")
                nc.gpsimd.memset(x3[:], 0.0)  # zeroes the v32 pads
                TC = 16 * V  # 400 cols = 16 t's per chunk
                for k in range(4):
                    Yc = psc.tile([C, TC], f32, tag="chunk", name="chunk")
                    nc.tensor.matmul(Yc[:], w["wT_tada"][:],
                                     X[:, k * TC:(k + 1) * TC],
                                     start=True, stop=True)
                    t0, t1 = 16 * k, 16 * (k + 1)
                    nc.vector.tensor_tensor(
                        x3[:].rearrange("c (t v) -> c t v", v=V32)[:, t0:t1, 0:V],
                        Yc[:].rearrange("c (t v) -> c t v", v=V),
                        alpha_rf[:, t0:t1].unsqueeze(2).broadcast_to((O, 16, V)),
                        op=ALU.mult)
                X3v = x3vpool.tile([C, TV32], f32, tag="X3v", name="X3v")
                nc.vector.transpose(X3v[:], x3[:])

                # ---- m[c, (u,v)] = alpha*(conv4 @ D + b4) + A ----
                # m32 is (u32, v32)-padded: pad u-slots/v-lanes are zero so the
                # graph-conv matmuls can use full 32x32 weight tiles (the pad
                # rows of each PSUM block compute to zero, keeping every read
                # byte defined).
                m32 = mpool.tile([O, U32V32], f32, tag="m32", name="m32")
                nc.gpsimd.memset(m32[:], 0.0)
                for (u0, u1) in ((0, 16), (16, V)):
                    mc = psc.tile([C, TC], f32, tag="chunk", name="chunk")
                    nc.tensor.matmul(mc[:, 0:(u1 - u0) * V], w["lhsT18"][:],
                                     D18[:, u0 * V:u1 * V], start=True, stop=True)
                    nc.scalar.copy(
                        m32[:].rearrange("c (u v) -> c u v", v=V32)
                        [:, u0:u1, 0:V],
                        mc[:, 0:(u1 - u0) * V].rearrange("c (u v) -> c u v", v=V))
                mv = mvpool.tile([C, U32V32], f32, tag="mv", name="mv")
                nc.vector.transpose(mv[:], m32[:])
                pend_b = (X3v, mv, n)

                # ---- stage C: finish the sample from two iterations back ----
                if pend_c is not None:
                    emit_tail(*pend_c)
                pend_c = new_c

            # drain the pipeline
            new_c = (emit_waves(*pend_b), pend_b[2])
            if pend_c is not None:
                emit_tail(*pend_c)
            emit_tail(*new_c)

    nc.compile()
    return nc


def _fold_weights(A, conv1_w, conv1_b, conv2_w, conv2_b, conv4_w, conv4_b,
                  rf_g_w, rf_g_b, rf_a_w, rf_a_b, bn_gamma, bn_beta,
                  rf_b_w, tada_w, alpha):
    af = float(np.asarray(alpha))
    f = np.float32
    s = (bn_gamma / np.sqrt(1.0 + BN_EPS)).astype(f)
    rf_a_w2 = (rf_a_w * s[:, None, None]).astype(f)
    rf_ab2 = (rf_a_b * s + bn_beta).astype(f)
    lhsT18 = np.concatenate([
        af * conv4_w.T.astype(f),            # (16, 128)
        af * conv4_b[None, :].astype(f),     # (1, 128)
        np.ones((1, O), f),
    ], axis=0)
    d18c = np.stack([np.ones(UV, f), A.astype(f).reshape(UV)], axis=0)
    return {
        "wT_tada": np.ascontiguousarray(tada_w.T.astype(f)),
        "rf_gT": np.ascontiguousarray((rf_g_w.T / (T * V)).astype(f)),
        "rf_g_b": rf_g_b.astype(f).reshape(C, 1),
        "w1T": np.ascontiguousarray((conv1_w.T / T).astype(f)),
        "b1": conv1_b.astype(f).reshape(R, 1),
        "w2T": np.ascontiguousarray((conv2_w.T / T).astype(f)),
        "b2": conv2_b.astype(f).reshape(R, 1),
        "rf_aT": np.concatenate([rf_a_w2[:, :, k].T for k in range(3)], axis=1),
        "rf_ab": rf_ab2.reshape(CH, 1),
        "rf_bT": np.concatenate([rf_b_w[:, :, k].T.astype(f) for k in range(3)],
                                axis=1),
        "lhsT18": lhsT18,
        "d18c": d18c,
    }


def _neff_io(nc):
    """(in_names, out_names, out_avals, out_shapes, partition_name)."""
    import jax
    import concourse.mybir as mybir

    partition_name = (nc.partition_id_tensor.name
                      if nc.partition_id_tensor else None)
    in_names, out_names, out_avals, out_shapes = [], [], [], []
    for alloc in nc.m.functions[0].allocations:
        if not isinstance(alloc, mybir.MemoryLocationSet):
            continue
        name = alloc.memorylocations[0].name
        if alloc.kind == "ExternalInput":
            if name != partition_name:
                in_names.append(name)
        elif alloc.kind == "ExternalOutput":
            out_names.append(name)
            shape = tuple(alloc.tensor_shape)
            dtype = mybir.dt.np(alloc.dtype)
            out_avals.append(jax.core.ShapedArray(shape, dtype))
            out_shapes.append((shape, dtype))
    return in_names, out_names, out_avals, out_shapes, partition_name


def _make_runner(nc):
    """Cached jitted SPMD executable (mirrors bass2jax.run_bass_via_pjrt)."""
    import jax
    from jax.sharding import Mesh, PartitionSpec
    from jax.experimental.shard_map import shard_map
    from concourse import bass2jax

    bass2jax.install_neuronx_cc_hook()
    assert nc.dbg_addr is None
    in_names, out_names, out_avals, out_shapes, partition_name = _neff_io(nc)
    n_params = len(in_names)
    all_in_names = tuple(in_names) + tuple(out_names)
    if partition_name is not None:
        all_in_names = all_in_names + (partition_name,)

    def _body(*args):
        operands = list(args)
        if partition_name is not None:
            operands.append(bass2jax.partition_id_tensor())
        outs = bass2jax._bass_exec_p.bind(
            *operands, out_avals=tuple(out_avals), in_names=all_in_names,
            out_names=tuple(out_names), lowering_input_output_aliases=(),
            sim_require_finite=True, sim_require_nnan=True, nc=nc)
        return tuple(outs)

    devices = jax.devices()[:N_CORES]
    mesh = Mesh(np.asarray(devices), ("core",))
    n_outs = len(out_names)
    # No donation: the kernel writes every element of every output, so the
    # zero placeholders can stay device-resident and be reused across calls.
    sharded = jax.jit(
        shard_map(_body, mesh=mesh,
                  in_specs=(PartitionSpec("core"),) * (n_params + n_outs),
                  out_specs=(PartitionSpec("core"),) * n_outs,
                  check_rep=False),
        keep_unused=True)
    zeros_dev = [jax.device_put(np.zeros((N_CORES * s[0], *s[1:]), d))
                 for s, d in out_shapes]
    return sharded, in_names, out_names, out_shapes, zeros_dev


def _prepare_concat_inputs(x, wmap, in_names):
    """Global (n_cores*dim0, ...) arrays in the NEFF's input order."""
    per = {"xs": np.ascontiguousarray(x, dtype=np.float32)}
    for k, v in wmap.items():
        per[k] = np.concatenate([v[None]] * N_CORES, axis=0).reshape(
            N_CORES * v.shape[0], *v.shape[1:])
    return [per[nm] for nm in in_names]


def kernel(x, A, conv1_w, conv1_b, conv2_w, conv2_b, conv4_w, conv4_b,
           rf_g_w, rf_g_b, rf_a_w, rf_a_b, bn_gamma, bn_beta,
           rf_b_w, tada_w, alpha):
    if "nc" not in _CACHE:
        _CACHE["nc"] = _build_program()
        _CACHE["runner"] = _make_runner(_CACHE["nc"])
    sharded, in_names, out_names, out_shapes, zeros_dev = _CACHE["runner"]

    wmap = _fold_weights(A, conv1_w, conv1_b, conv2_w, conv2_b, conv4_w,
                         conv4_b, rf_g_w, rf_g_b, rf_a_w, rf_a_b, bn_gamma,
                         bn_beta, rf_b_w, tada_w, alpha)
    wmap = {k: np.ascontiguousarray(v, dtype=np.float32) for k, v in wmap.items()}

    ins = _prepare_concat_inputs(x, wmap, in_names)
    outs = sharded(*ins, *zeros_dev)
    i = out_names.index("out")
    return np.asarray(outs[i])
